# revision 2
# baseline (speedup 1.0000x reference)
"""Trainium2 Bass kernel v2 for the dendritic template-gated FFN.

Math (token n, output feature h; W=16 windows of K=64 input features):
    s[n,h,w] = <x[n, w*64:(w+1)*64], template[h, w*64:(w+1)*64]>
    out[n,h] = sum_w softmax_w(s) * silu(s) = [sum_w s*z_w] / [sum_w e_w]
    with z = e^s - sigma(s) (since e^s*silu(s) = s*(e^s - sigma(s))).

Restructuring vs baseline (which was ACT-bound: exp+tanh+2 DVE/Pool products
and 3 PE reduction streams):
    e2  = 2*e^s = Exp(s + ln2)            (ACT pass 1)
    th  = tanh(s/2)                        (ACT pass 2, same table set)
  z-form slots:
    z'' = e2 - th                          (DVE TT fp16 2x)
    p   = s * z''                          (DVE stt, PSUM fp32 x fp16)
    num'' += I@p  and  -S_kb               (S = per-k-block matmul with -x)
  q-form slots (Pool gets off-critical-chain work):
    mh  = (s*0.5)*e2 = s*e^s               (DVE stt; frees psum slot early)
    q   = th * mh                          (Pool TT fp16, all SBUF)
    num'' += I@mh + I@q                    (mh(1+th) = e2*silu)
    den'' += I@e2   for all slots;  out = num'' * recip_fast(den'')
since s*z''-s = 2*s*z = e2*silu.  num''=2num, den''=2den -> exact ratio.

Pipeline: psum = 3-slot pool of [128,1024] s tiles + one [128,1024] den|num
tile. Per slot: 2 PE matmuls -> ACT exp -> ACT tanh -> slot-freeing DVE
product -> deferred PE reduction matmuls (LAG slots later, emitted ahead of
the next matmuls so the PE FIFO never head-blocks). Prologue transposes for
i>0 / j>0 are spread one piece per pair through the main loop.

Sharding: data-parallel over tokens, 512 per NeuronCore x 8 cores.
"""

import numpy as np
from contextlib import ExitStack

import concourse.bass as bass
import concourse.bacc as bacc
import concourse.mybir as mybir
import concourse.tile as tile
from concourse.bass_utils import run_bass_kernel_spmd

AF = mybir.ActivationFunctionType
ALU = mybir.AluOpType
DT = mybir.dt

N_TOTAL = 4096
IN_F = 1024
OUT_F = 2048
WIN = 64
NW = 16
N_CORES = 8
N_SH = N_TOTAL // N_CORES   # 512 tokens per core
QFORM = [True, True, False, False]   # slot form pattern (True -> q-form)
LAG = 4
LN2 = float(np.log(2.0))


def build_program(n_tok=N_SH):
    nc = bacc.Bacc(
        "TRN2",
        target_bir_lowering=False,
        debug=False,
        enable_asserts=False,
        num_devices=N_CORES,
    )
    x_d = nc.dram_tensor("x", [n_tok, IN_F], DT.float32, kind="ExternalInput").ap()
    t_d = nc.dram_tensor(
        "template_flat", [OUT_F, IN_F], DT.float32, kind="ExternalInput"
    ).ap()
    eye_d = nc.dram_tensor("eye", [128, 128], DT.float32, kind="ExternalInput").ap()
    out_d = nc.dram_tensor("out", [n_tok, OUT_F], DT.float32, kind="ExternalOutput").ap()

    NT = n_tok // 128       # 4 token tiles
    NJ = OUT_F // 512       # 4 h chunks
    KB = IN_F // 128        # 8 k-blocks (2 windows each)
    HB = OUT_F // 128       # 16 h blocks of template

    with ExitStack() as ctx:
        tc = ctx.enter_context(tile.TileContext(nc))

        const_pool = ctx.enter_context(tc.tile_pool(name="const", bufs=1))
        eye_t = const_pool.tile([128, 128], DT.float32, tag="eye")
        nc.sync.dma_start(eye_t[:], eye_d[:])
        eye_h = const_pool.tile([128, 128], DT.float16, tag="eyeh")
        nc.vector.tensor_copy(eye_h[:], eye_t[:])
        ln2_t = const_pool.tile([128, 1], DT.float32, tag="ln2")
        nc.gpsimd.memset(ln2_t[:], LN2)

        persist = ctx.enter_context(tc.tile_pool(name="persist", bufs=1))
        xT = [persist.tile([128, n_tok], DT.float16, tag=f"xT{kb}", name=f"xT{kb}")
              for kb in range(KB)]
        xTn = [persist.tile([128, n_tok], DT.float16, tag=f"xTn{kb}", name=f"xTn{kb}")
               for kb in range(KB)]
        tT = [persist.tile([128, OUT_F], DT.float16, tag=f"tT{kb}", name=f"tT{kb}")
              for kb in range(KB)]

        # staging tiles stay open through the main loop (late transposes)
        stage = ctx.enter_context(tc.tile_pool(name="stage", bufs=1))
        t_nm = [stage.tile([128, IN_F], DT.float32, tag=f"tnm{hb}",
                           name=f"tnm{hb}") for hb in range(HB)]
        x_nm = [stage.tile([128, IN_F], DT.float32, tag=f"xnm{i}",
                           name=f"xnm{i}") for i in range(NT)]
        x_re = x_d.rearrange("(i p) k -> p i k", p=128)
        t_re = t_d.rearrange("(h p) k -> p h k", p=128)
        for hb in range(4):
            nc.sync.dma_start(t_nm[hb][:], t_re[:, hb, :])
        for i in range(NT):
            nc.sync.dma_start(x_nm[i][:], x_re[:, i, :])
        for hb in range(4, HB):
            nc.sync.dma_start(t_nm[hb][:], t_re[:, hb, :])

        # transpose piece emitters; `ps_fn()` yields a [128,1024] psum tile
        def do_t_chunk(hq, kb, ps_fn):
            """template rows hq*128..(hq+4)*128, k-block kb -> tT[kb]."""
            ps = ps_fn()
            for q in range(4):
                nc.tensor.transpose(
                    ps[:, q * 128:(q + 1) * 128],
                    t_nm[hq + q][:, kb * 128:(kb + 1) * 128],
                    eye_t[:],
                )
            nc.vector.tensor_copy(tT[kb][:, hq * 128:(hq + 4) * 128],
                                  ps[:, 0:512])

        def do_x_piece(i, kb2, ps_fn):
            """x token-tile i, k-blocks kb2..kb2+3 -> xT/xTn cols."""
            ps = ps_fn()
            for kk in range(4):
                kb = kb2 + kk
                nc.tensor.transpose(
                    ps[:, kk * 128:(kk + 1) * 128],
                    x_nm[i][:, kb * 128:(kb + 1) * 128],
                    eye_t[:],
                )
            sl = slice(i * 128, (i + 1) * 128)
            for kk in range(4):
                kb = kb2 + kk
                nc.vector.tensor_copy(xT[kb][:, sl],
                                      ps[:, kk * 128:(kk + 1) * 128])
                nc.vector.tensor_scalar(xTn[kb][:, sl], xT[kb][:, sl],
                                        -1.0, None, ALU.mult)

        # ---- pre-loop: only what region (0,0) needs ----
        with tc.tile_pool(name="tpsum", bufs=2, space="PSUM") as tpsum:
            def pre_ps():
                return tpsum.tile([128, 1024], DT.float32, tag="tp", name="tp")
            for kb in range(KB):
                do_t_chunk(0, kb, pre_ps)      # tT[:, 0:512]  (j=0)
            do_x_piece(0, 0, pre_ps)           # xT/xTn[:, 0:128] (i=0)
            do_x_piece(0, 4, pre_ps)

        # ---- main pools ----
        spool = ctx.enter_context(tc.tile_pool(name="spsum", bufs=3, space="PSUM"))
        dnpool = ctx.enter_context(tc.tile_pool(name="dnpsum", bufs=1, space="PSUM"))
        e_pool = ctx.enter_context(tc.tile_pool(name="epool", bufs=6))
        th_pool = ctx.enter_context(tc.tile_pool(name="thpool", bufs=6))
        z_pool = ctx.enter_context(tc.tile_pool(name="zpool", bufs=6))
        p_pool = ctx.enter_context(tc.tile_pool(name="ppool", bufs=6))
        tail_pool = ctx.enter_context(tc.tile_pool(name="tail", bufs=2))

        def ring_ps():
            return spool.tile([128, 1024], DT.float32, tag="s", name="s")

        # late prologue pieces, one per pair of main-loop groups
        late_pieces = []
        for i in range(1, NT):
            for kb2 in (0, 4):
                late_pieces.append(
                    lambda i=i, kb2=kb2: do_x_piece(i, kb2, ring_ps))
        for hq in range(4, HB, 4):
            for kb in range(KB):
                late_pieces.append(
                    lambda hq=hq, kb=kb: do_t_chunk(hq, kb, ring_ps))
        late_pieces.reverse()

        pending = []

        def emit_pending(keep):
            while len(pending) > keep:
                pending.pop(0)()

        for j in range(NJ):
            for i in range(NT):
                r = j * NT + i
                dn = dnpool.tile([128, 1024], DT.float32, tag="dn")
                den = dn[:, 0:512]
                num = dn[:, 512:1024]

                # per-slot form: True -> q-form (mh=s*e DVE, q=th*mh Pool,
                # 3 num mms, no S); False -> z-form (z=e2-th, p=s*z DVE,
                # 2 num mms + S-correction matmul for that k-block)
                forms = [QFORM[(r * 8 + g) % len(QFORM)] for g in range(8)]

                def s_task(j=j, i=i, num=num, forms=forms):
                    # executes after red_task(g=0) (which carries num's
                    # start=True) and before red_task(g=7) (num's stop);
                    # only z-form k-blocks need the -S correction
                    for kb in range(KB):
                        if forms[kb]:
                            continue
                        nc.tensor.matmul(
                            num,
                            xTn[kb][:, i * 128:(i + 1) * 128],
                            tT[kb][:, j * 512:(j + 1) * 512],
                            start=False, stop=False,
                            skip_group_check=True,
                        )

                for g in range(8):
                    # emit deferred reductions BEFORE this group's matmuls so
                    # they aren't FIFO-blocked behind a slot-starved matmul
                    emit_pending(LAG)
                    st = spool.tile([128, 1024], DT.float32, tag="s")
                    for widx in range(2):
                        w = g * 2 + widx
                        base = (w % 2) * 64
                        lhsT = xT[w // 2][base:base + 64,
                                          i * 128:(i + 1) * 128]
                        rhs = tT[w // 2][base:base + 64,
                                         j * 512:(j + 1) * 512]
                        nc.tensor.matmul(
                            st[:, widx * 512:(widx + 1) * 512],
                            lhsT, rhs,
                            start=True, stop=True, skip_group_check=True,
                        )

                    e_t = e_pool.tile([128, 1024], DT.float16, tag="e")
                    th_t = th_pool.tile([128, 1024], DT.float16, tag="th")
                    nc.scalar.activation(e_t[:], st[:], AF.Exp, bias=ln2_t[:])
                    nc.scalar.activation(th_t[:], st[:], AF.Tanh, scale=0.5)
                    qform = forms[g]
                    if qform:
                        # mh = s*e = (s*0.5)*e2 ; frees slot w/o waiting z
                        p_t = p_pool.tile([128, 1024], DT.float16, tag="p")
                        nc.vector.scalar_tensor_tensor(
                            p_t[:], st[:], 0.5, e_t[:], ALU.mult, ALU.mult,
                        )
                        q_t = z_pool.tile([128, 1024], DT.float16, tag="z")
                        nc.gpsimd.tensor_tensor(q_t[:], th_t[:], p_t[:],
                                                ALU.mult)
                    else:
                        z_t = z_pool.tile([128, 1024], DT.float16, tag="z")
                        nc.vector.tensor_tensor(z_t[:], e_t[:], th_t[:],
                                                ALU.subtract)
                        p_t = p_pool.tile([128, 1024], DT.float16, tag="p")
                        nc.vector.scalar_tensor_tensor(
                            p_t[:], st[:], 1.0, z_t[:], ALU.mult, ALU.mult,
                        )
                        q_t = None

                    def red_task(g=g, den=den, num=num, e_t=e_t, p_t=p_t,
                                 q_t=q_t):
                        for c in range(2):
                            w = g * 2 + c
                            sl = slice(c * 512, (c + 1) * 512)
                            nc.tensor.matmul(
                                den, eye_h[:], e_t[:, sl],
                                start=(w == 0), stop=(w == NW - 1),
                                skip_group_check=True,
                            )
                            nc.tensor.matmul(
                                num, eye_h[:], p_t[:, sl],
                                start=(w == 0),
                                stop=(w == NW - 1 and q_t is None),
                                skip_group_check=True,
                            )
                            if q_t is not None:
                                nc.tensor.matmul(
                                    num, eye_h[:], q_t[:, sl],
                                    start=False, stop=(w == NW - 1),
                                    skip_group_check=True,
                                )
                    pending.append(red_task)
                    if g == 1:
                        # lands after red_task(g=0), which carries num's
                        # start=True; before red_task(g=7), which stops it
                        pending.append(s_task)
                    if late_pieces and g % 2 == 1:
                        late_pieces.pop()()

                def tail_task(j=j, i=i, dn=dn, den=den, num=num):
                    r_t = tail_pool.tile([128, 512], DT.float32, tag="r")
                    nc.vector.reciprocal_approx_fast(r_t[:], den)
                    o_t = tail_pool.tile([128, 512], DT.float32, tag="o")
                    nc.vector.tensor_tensor(o_t[:], num, r_t[:], ALU.mult)
                    nc.sync.dma_start(
                        out_d[i * 128:(i + 1) * 128, j * 512:(j + 1) * 512],
                        o_t[:],
                    )
                pending.append(tail_task)
        emit_pending(0)

    nc.compile()
    return nc


_EYE = None
_PROG = None


def _eye_input():
    global _EYE
    if _EYE is None:
        _EYE = np.eye(128, dtype=np.float32)
    return _EYE


def kernel(x: np.ndarray, template_flat: np.ndarray) -> np.ndarray:
    global _PROG
    x = np.ascontiguousarray(x, dtype=np.float32)
    template_flat = np.ascontiguousarray(template_flat, dtype=np.float32)
    assert x.shape == (N_TOTAL, IN_F) and template_flat.shape == (OUT_F, IN_F)
    if _PROG is None:
        _PROG = build_program()
    eye = _eye_input()
    in_maps = [
        {
            "x": x[c * N_SH:(c + 1) * N_SH],
            "template_flat": template_flat,
            "eye": eye,
        }
        for c in range(N_CORES)
    ]
    res = run_bass_kernel_spmd(_PROG, in_maps, core_ids=list(range(N_CORES)))
    return np.concatenate([r["out"] for r in res.results], axis=0)


# revision 3
# speedup vs baseline: 1.0137x; 1.0137x over previous
"""Trainium2 Bass kernel v2 for the dendritic template-gated FFN.

Math (token n, output feature h; W=16 windows of K=64 input features):
    s[n,h,w] = <x[n, w*64:(w+1)*64], template[h, w*64:(w+1)*64]>
    out[n,h] = sum_w softmax_w(s) * silu(s) = [sum_w s*z_w] / [sum_w e_w]
    with z = e^s - sigma(s) (since e^s*silu(s) = s*(e^s - sigma(s))).

Restructuring vs baseline (which was ACT-bound: exp+tanh+2 DVE/Pool products
and 3 PE reduction streams):
    e2  = 2*e^s = Exp(s + ln2)            (ACT pass 1)
    th  = tanh(s/2)                        (ACT pass 2, same table set)
  z-form slots:
    z'' = e2 - th                          (DVE TT fp16 2x)
    p   = s * z''                          (DVE stt, PSUM fp32 x fp16)
    num'' += I@p  and  -S_kb               (S = per-k-block matmul with -x)
  q-form slots (Pool gets off-critical-chain work):
    mh  = (s*0.5)*e2 = s*e^s               (DVE stt; frees psum slot early)
    q   = th * mh                          (Pool TT fp16, all SBUF)
    num'' += I@mh + I@q                    (mh(1+th) = e2*silu)
    den'' += I@e2   for all slots;  out = num'' * recip_fast(den'')
since s*z''-s = 2*s*z = e2*silu.  num''=2num, den''=2den -> exact ratio.

Pipeline: psum = 3-slot pool of [128,1024] s tiles + one [128,1024] den|num
tile. Per slot: 2 PE matmuls -> ACT exp -> ACT tanh -> slot-freeing DVE
product -> deferred PE reduction matmuls (LAG slots later, emitted ahead of
the next matmuls so the PE FIFO never head-blocks). Prologue transposes for
i>0 / j>0 are spread one piece per pair through the main loop.

Sharding: data-parallel over tokens, 512 per NeuronCore x 8 cores.
"""

import numpy as np
from contextlib import ExitStack

import concourse.bass as bass
import concourse.bacc as bacc
import concourse.mybir as mybir
import concourse.tile as tile
from concourse.bass_utils import run_bass_kernel_spmd

AF = mybir.ActivationFunctionType
ALU = mybir.AluOpType
DT = mybir.dt

N_TOTAL = 4096
IN_F = 1024
OUT_F = 2048
WIN = 64
NW = 16
N_CORES = 8
N_SH = N_TOTAL // N_CORES   # 512 tokens per core
QFORM = [True, True, False, False]   # slot form pattern (True -> q-form)
LAG = 4
LN2 = float(np.log(2.0))


def build_program(n_tok=N_SH):
    nc = bacc.Bacc(
        "TRN2",
        target_bir_lowering=False,
        debug=False,
        enable_asserts=False,
        num_devices=N_CORES,
    )
    x_d = nc.dram_tensor("x", [n_tok, IN_F], DT.float32, kind="ExternalInput").ap()
    t_d = nc.dram_tensor(
        "template_flat", [OUT_F, IN_F], DT.float32, kind="ExternalInput"
    ).ap()
    eye_d = nc.dram_tensor("eye", [128, 128], DT.float32, kind="ExternalInput").ap()
    out_d = nc.dram_tensor("out", [n_tok, OUT_F], DT.float32, kind="ExternalOutput").ap()

    NT = n_tok // 128       # 4 token tiles
    NJ = OUT_F // 512       # 4 h chunks
    KB = IN_F // 128        # 8 k-blocks (2 windows each)
    HB = OUT_F // 128       # 16 h blocks of template

    with ExitStack() as ctx:
        tc = ctx.enter_context(tile.TileContext(nc))

        const_pool = ctx.enter_context(tc.tile_pool(name="const", bufs=1))
        eye_t = const_pool.tile([128, 128], DT.float32, tag="eye")
        nc.sync.dma_start(eye_t[:], eye_d[:])
        eye_h = const_pool.tile([128, 128], DT.float16, tag="eyeh")
        nc.vector.tensor_copy(eye_h[:], eye_t[:])
        ln2_t = const_pool.tile([128, 1], DT.float32, tag="ln2")
        nc.gpsimd.memset(ln2_t[:], LN2)

        persist = ctx.enter_context(tc.tile_pool(name="persist", bufs=1))
        xT = [persist.tile([128, n_tok], DT.float16, tag=f"xT{kb}", name=f"xT{kb}")
              for kb in range(KB)]
        xTn = [persist.tile([128, n_tok], DT.float16, tag=f"xTn{kb}", name=f"xTn{kb}")
               for kb in range(KB)]
        tT = [persist.tile([128, OUT_F], DT.float16, tag=f"tT{kb}", name=f"tT{kb}")
              for kb in range(KB)]

        # staging tiles stay open through the main loop (late transposes)
        stage = ctx.enter_context(tc.tile_pool(name="stage", bufs=1))
        t_nm = [stage.tile([128, IN_F], DT.float32, tag=f"tnm{hb}",
                           name=f"tnm{hb}") for hb in range(HB)]
        x_nm = [stage.tile([128, IN_F], DT.float32, tag=f"xnm{i}",
                           name=f"xnm{i}") for i in range(NT)]
        x_re = x_d.rearrange("(i p) k -> p i k", p=128)
        t_re = t_d.rearrange("(h p) k -> p h k", p=128)
        # region (0,0)'s inputs first: t rows 0-511 and x tile 0
        for hb in range(2):
            nc.sync.dma_start(t_nm[hb][:], t_re[:, hb, :])
        nc.sync.dma_start(x_nm[0][:], x_re[:, 0, :])
        for hb in range(2, 4):
            nc.sync.dma_start(t_nm[hb][:], t_re[:, hb, :])
        for i in range(1, NT):
            nc.sync.dma_start(x_nm[i][:], x_re[:, i, :])
        for hb in range(4, HB):
            nc.sync.dma_start(t_nm[hb][:], t_re[:, hb, :])

        # transpose piece emitters; `ps_fn()` yields a [128,1024] psum tile
        def do_t_chunk(hq, kb, ps_fn):
            """template rows hq*128..(hq+4)*128, k-block kb -> tT[kb]."""
            ps = ps_fn()
            for q in range(4):
                nc.tensor.transpose(
                    ps[:, q * 128:(q + 1) * 128],
                    t_nm[hq + q][:, kb * 128:(kb + 1) * 128],
                    eye_t[:],
                )
            nc.vector.tensor_copy(tT[kb][:, hq * 128:(hq + 4) * 128],
                                  ps[:, 0:512])

        def do_x_piece(i, kb2, ps_fn):
            """x token-tile i, k-blocks kb2..kb2+3 -> xT/xTn cols."""
            ps = ps_fn()
            for kk in range(4):
                kb = kb2 + kk
                nc.tensor.transpose(
                    ps[:, kk * 128:(kk + 1) * 128],
                    x_nm[i][:, kb * 128:(kb + 1) * 128],
                    eye_t[:],
                )
            sl = slice(i * 128, (i + 1) * 128)
            for kk in range(4):
                kb = kb2 + kk
                nc.vector.tensor_copy(xT[kb][:, sl],
                                      ps[:, kk * 128:(kk + 1) * 128])
                nc.vector.tensor_scalar(xTn[kb][:, sl], xT[kb][:, sl],
                                        -1.0, None, ALU.mult)

        # ---- pre-loop: only what region (0,0)'s first groups need ----
        with tc.tile_pool(name="tpsum", bufs=2, space="PSUM") as tpsum:
            def pre_ps():
                return tpsum.tile([128, 1024], DT.float32, tag="tp", name="tp")
            do_t_chunk(0, 0, pre_ps)           # tT[0][:, 0:512]
            do_x_piece(0, 0, pre_ps)           # xT/xTn[:, 0:128] kb 0-3
            do_t_chunk(0, 1, pre_ps)
            do_x_piece(0, 4, pre_ps)           # xT/xTn kb 4-7
            do_t_chunk(0, 2, pre_ps)
            do_t_chunk(0, 3, pre_ps)

        # ---- main pools ----
        spool = ctx.enter_context(tc.tile_pool(name="spsum", bufs=3, space="PSUM"))
        dnpool = ctx.enter_context(tc.tile_pool(name="dnpsum", bufs=1, space="PSUM"))
        e_pool = ctx.enter_context(tc.tile_pool(name="epool", bufs=6))
        th_pool = ctx.enter_context(tc.tile_pool(name="thpool", bufs=6))
        z_pool = ctx.enter_context(tc.tile_pool(name="zpool", bufs=8))
        p_pool = ctx.enter_context(tc.tile_pool(name="ppool", bufs=8))
        tail_pool = ctx.enter_context(tc.tile_pool(name="tail", bufs=2))

        def ring_ps():
            return spool.tile([128, 1024], DT.float32, tag="s", name="s")

        # region-0 just-in-time pieces: tT[g] must be emitted before group
        # g's matmuls (one piece at the top of each early group)
        jit_pieces = [lambda kb=kb: do_t_chunk(0, kb, ring_ps)
                      for kb in range(4, KB)]
        jit_pieces.reverse()

        # late prologue pieces, one per pair of main-loop groups
        late_pieces = []
        for i in range(1, NT):
            for kb2 in (0, 4):
                late_pieces.append(
                    lambda i=i, kb2=kb2: do_x_piece(i, kb2, ring_ps))
        for hq in range(4, HB, 4):
            for kb in range(KB):
                late_pieces.append(
                    lambda hq=hq, kb=kb: do_t_chunk(hq, kb, ring_ps))
        late_pieces.reverse()

        pending = []

        def emit_pending(keep):
            while len(pending) > keep:
                pending.pop(0)()

        for j in range(NJ):
            for i in range(NT):
                r = j * NT + i
                dn = dnpool.tile([128, 1024], DT.float32, tag="dn")
                den = dn[:, 0:512]
                num = dn[:, 512:1024]

                # per-slot form: True -> q-form (mh=s*e DVE, q=th*mh Pool,
                # 3 num mms, no S); False -> z-form (z=e2-th, p=s*z DVE,
                # 2 num mms + S-correction matmul for that k-block)
                forms = [QFORM[(r * 8 + g) % len(QFORM)] for g in range(8)]

                def s_task(j=j, i=i, num=num, forms=forms):
                    # executes after red_task(g=0) (which carries num's
                    # start=True) and before red_task(g=7) (num's stop);
                    # only z-form k-blocks need the -S correction
                    for kb in range(KB):
                        if forms[kb]:
                            continue
                        nc.tensor.matmul(
                            num,
                            xTn[kb][:, i * 128:(i + 1) * 128],
                            tT[kb][:, j * 512:(j + 1) * 512],
                            start=False, stop=False,
                            skip_group_check=True,
                        )

                for g in range(8):
                    if jit_pieces:
                        jit_pieces.pop()()
                    # emit deferred reductions BEFORE this group's matmuls so
                    # they aren't FIFO-blocked behind a slot-starved matmul
                    emit_pending(LAG)
                    st = spool.tile([128, 1024], DT.float32, tag="s")
                    for widx in range(2):
                        w = g * 2 + widx
                        base = (w % 2) * 64
                        lhsT = xT[w // 2][base:base + 64,
                                          i * 128:(i + 1) * 128]
                        rhs = tT[w // 2][base:base + 64,
                                         j * 512:(j + 1) * 512]
                        nc.tensor.matmul(
                            st[:, widx * 512:(widx + 1) * 512],
                            lhsT, rhs,
                            start=True, stop=True, skip_group_check=True,
                        )

                    e_t = e_pool.tile([128, 1024], DT.float16, tag="e")
                    th_t = th_pool.tile([128, 1024], DT.float16, tag="th")
                    nc.scalar.activation(e_t[:], st[:], AF.Exp, bias=ln2_t[:])
                    nc.scalar.activation(th_t[:], st[:], AF.Tanh, scale=0.5)
                    qform = forms[g]
                    if qform:
                        # mh = s*e = (s*0.5)*e2 ; frees slot w/o waiting z
                        p_t = p_pool.tile([128, 1024], DT.float16, tag="p")
                        nc.vector.scalar_tensor_tensor(
                            p_t[:], st[:], 0.5, e_t[:], ALU.mult, ALU.mult,
                        )
                        q_t = z_pool.tile([128, 1024], DT.float16, tag="z")
                        nc.gpsimd.tensor_tensor(q_t[:], th_t[:], p_t[:],
                                                ALU.mult)
                    else:
                        z_t = z_pool.tile([128, 1024], DT.float16, tag="z")
                        nc.vector.tensor_tensor(z_t[:], e_t[:], th_t[:],
                                                ALU.subtract)
                        p_t = p_pool.tile([128, 1024], DT.float16, tag="p")
                        nc.vector.scalar_tensor_tensor(
                            p_t[:], st[:], 1.0, z_t[:], ALU.mult, ALU.mult,
                        )
                        q_t = None

                    def red_task(g=g, den=den, num=num, e_t=e_t, p_t=p_t,
                                 q_t=q_t):
                        for c in range(2):
                            w = g * 2 + c
                            sl = slice(c * 512, (c + 1) * 512)
                            nc.tensor.matmul(
                                den, eye_h[:], e_t[:, sl],
                                start=(w == 0), stop=(w == NW - 1),
                                skip_group_check=True,
                            )
                            nc.tensor.matmul(
                                num, eye_h[:], p_t[:, sl],
                                start=(w == 0),
                                stop=(w == NW - 1 and q_t is None),
                                skip_group_check=True,
                            )
                            if q_t is not None:
                                nc.tensor.matmul(
                                    num, eye_h[:], q_t[:, sl],
                                    start=False, stop=(w == NW - 1),
                                    skip_group_check=True,
                                )
                    pending.append(red_task)
                    if g == 1:
                        # lands after red_task(g=0), which carries num's
                        # start=True; before red_task(g=7), which stops it
                        pending.append(s_task)
                    if late_pieces and g % 2 == 1:
                        late_pieces.pop()()

                def tail_task(j=j, i=i, dn=dn, den=den, num=num):
                    r_t = tail_pool.tile([128, 512], DT.float32, tag="r")
                    nc.vector.reciprocal_approx_fast(r_t[:], den)
                    o_t = tail_pool.tile([128, 512], DT.float32, tag="o")
                    nc.vector.tensor_tensor(o_t[:], num, r_t[:], ALU.mult)
                    nc.sync.dma_start(
                        out_d[i * 128:(i + 1) * 128, j * 512:(j + 1) * 512],
                        o_t[:],
                    )
                pending.append(tail_task)
        emit_pending(0)

    nc.compile()
    return nc


_EYE = None
_PROG = None


def _eye_input():
    global _EYE
    if _EYE is None:
        _EYE = np.eye(128, dtype=np.float32)
    return _EYE


def kernel(x: np.ndarray, template_flat: np.ndarray) -> np.ndarray:
    global _PROG
    x = np.ascontiguousarray(x, dtype=np.float32)
    template_flat = np.ascontiguousarray(template_flat, dtype=np.float32)
    assert x.shape == (N_TOTAL, IN_F) and template_flat.shape == (OUT_F, IN_F)
    if _PROG is None:
        _PROG = build_program()
    eye = _eye_input()
    in_maps = [
        {
            "x": x[c * N_SH:(c + 1) * N_SH],
            "template_flat": template_flat,
            "eye": eye,
        }
        for c in range(N_CORES)
    ]
    res = run_bass_kernel_spmd(_PROG, in_maps, core_ids=list(range(N_CORES)))
    return np.concatenate([r["out"] for r in res.results], axis=0)


# revision 4
# speedup vs baseline: 1.0144x; 1.0007x over previous
"""Trainium2 Bass kernel v2 for the dendritic template-gated FFN.

Math (token n, output feature h; W=16 windows of K=64 input features):
    s[n,h,w] = <x[n, w*64:(w+1)*64], template[h, w*64:(w+1)*64]>
    out[n,h] = sum_w softmax_w(s) * silu(s) = [sum_w s*z_w] / [sum_w e_w]
    with z = e^s - sigma(s) (since e^s*silu(s) = s*(e^s - sigma(s))).

Restructuring vs baseline (which was ACT-bound: exp+tanh+2 DVE/Pool products
and 3 PE reduction streams):
    e2  = 2*e^s = Exp(s + ln2)            (ACT pass 1)
    th  = tanh(s/2)                        (ACT pass 2, same table set)
  z-form slots:
    z'' = e2 - th                          (DVE TT fp16 2x)
    p   = s * z''                          (DVE stt, PSUM fp32 x fp16)
    num'' += I@p  and  -S_kb               (S = per-k-block matmul with -x)
  q-form slots (Pool gets off-critical-chain work):
    mh  = (s*0.5)*e2 = s*e^s               (DVE stt; frees psum slot early)
    q   = th * mh                          (Pool TT fp16, all SBUF)
    num'' += I@mh + I@q                    (mh(1+th) = e2*silu)
    den'' += I@e2   for all slots;  out = num'' * recip_fast(den'')
since s*z''-s = 2*s*z = e2*silu.  num''=2num, den''=2den -> exact ratio.

Pipeline: psum = 3-slot pool of [128,1024] s tiles + one [128,1024] den|num
tile. Per slot: 2 PE matmuls -> ACT exp -> ACT tanh -> slot-freeing DVE
product -> deferred PE reduction matmuls (LAG slots later, emitted ahead of
the next matmuls so the PE FIFO never head-blocks). Prologue transposes for
i>0 / j>0 are spread one piece per pair through the main loop.

Sharding: data-parallel over tokens, 512 per NeuronCore x 8 cores.
"""

import numpy as np
from contextlib import ExitStack

import concourse.bass as bass
import concourse.bacc as bacc
import concourse.mybir as mybir
import concourse.tile as tile
from concourse.bass_utils import run_bass_kernel_spmd

AF = mybir.ActivationFunctionType
ALU = mybir.AluOpType
DT = mybir.dt

N_TOTAL = 4096
IN_F = 1024
OUT_F = 2048
WIN = 64
NW = 16
N_CORES = 8
N_SH = N_TOTAL // N_CORES   # 512 tokens per core
QFORM = [True, True, False, False]   # slot form pattern (True -> q-form)
LAG = 4
LN2 = float(np.log(2.0))


def build_program(n_tok=N_SH):
    nc = bacc.Bacc(
        "TRN2",
        target_bir_lowering=False,
        debug=False,
        enable_asserts=False,
        num_devices=N_CORES,
    )
    x_d = nc.dram_tensor("x", [n_tok, IN_F], DT.float32, kind="ExternalInput").ap()
    t_d = nc.dram_tensor(
        "template_flat", [OUT_F, IN_F], DT.float32, kind="ExternalInput"
    ).ap()
    eye_d = nc.dram_tensor("eye", [128, 128], DT.float32, kind="ExternalInput").ap()
    out_d = nc.dram_tensor("out", [n_tok, OUT_F], DT.float32, kind="ExternalOutput").ap()

    NT = n_tok // 128       # 4 token tiles
    NJ = OUT_F // 512       # 4 h chunks
    KB = IN_F // 128        # 8 k-blocks (2 windows each)
    HB = OUT_F // 128       # 16 h blocks of template

    with ExitStack() as ctx:
        tc = ctx.enter_context(tile.TileContext(nc))

        const_pool = ctx.enter_context(tc.tile_pool(name="const", bufs=1))
        eye_t = const_pool.tile([128, 128], DT.float32, tag="eye")
        nc.sync.dma_start(eye_t[:], eye_d[:])
        eye_h = const_pool.tile([128, 128], DT.float16, tag="eyeh")
        nc.vector.tensor_copy(eye_h[:], eye_t[:])
        ln2_t = const_pool.tile([128, 1], DT.float32, tag="ln2")
        nc.gpsimd.memset(ln2_t[:], LN2)

        persist = ctx.enter_context(tc.tile_pool(name="persist", bufs=1))
        xT = [persist.tile([128, n_tok], DT.float16, tag=f"xT{kb}", name=f"xT{kb}")
              for kb in range(KB)]
        xTn = [persist.tile([128, n_tok], DT.float16, tag=f"xTn{kb}", name=f"xTn{kb}")
               for kb in range(KB)]
        tT = [persist.tile([128, OUT_F], DT.float16, tag=f"tT{kb}", name=f"tT{kb}")
              for kb in range(KB)]

        # staging tiles stay open through the main loop (late transposes)
        stage = ctx.enter_context(tc.tile_pool(name="stage", bufs=1))
        t_nm = [stage.tile([128, IN_F], DT.float32, tag=f"tnm{hb}",
                           name=f"tnm{hb}") for hb in range(HB)]
        x_nm = [stage.tile([128, IN_F], DT.float32, tag=f"xnm{i}",
                           name=f"xnm{i}") for i in range(NT)]
        x_re = x_d.rearrange("(i p) k -> p i k", p=128)
        t_re = t_d.rearrange("(h p) k -> p h k", p=128)
        # region (0,0)'s inputs first: t rows 0-511 and x tile 0
        for hb in range(2):
            nc.sync.dma_start(t_nm[hb][:], t_re[:, hb, :])
        nc.sync.dma_start(x_nm[0][:], x_re[:, 0, :])
        for hb in range(2, 4):
            nc.sync.dma_start(t_nm[hb][:], t_re[:, hb, :])
        for i in range(1, NT):
            nc.sync.dma_start(x_nm[i][:], x_re[:, i, :])
        for hb in range(4, HB):
            nc.sync.dma_start(t_nm[hb][:], t_re[:, hb, :])

        # transpose piece emitters; `ps_fn()` yields a [128,1024] psum tile
        def do_t_chunk(hq, kb, ps_fn):
            """template rows hq*128..(hq+4)*128, k-block kb -> tT[kb]."""
            ps = ps_fn()
            for q in range(4):
                nc.tensor.transpose(
                    ps[:, q * 128:(q + 1) * 128],
                    t_nm[hq + q][:, kb * 128:(kb + 1) * 128],
                    eye_t[:],
                )
            nc.vector.tensor_copy(tT[kb][:, hq * 128:(hq + 4) * 128],
                                  ps[:, 0:512])

        def do_x_piece(i, kb2, ps_fn):
            """x token-tile i, k-blocks kb2..kb2+3 -> xT/xTn cols."""
            ps = ps_fn()
            for kk in range(4):
                kb = kb2 + kk
                nc.tensor.transpose(
                    ps[:, kk * 128:(kk + 1) * 128],
                    x_nm[i][:, kb * 128:(kb + 1) * 128],
                    eye_t[:],
                )
            sl = slice(i * 128, (i + 1) * 128)
            for kk in range(4):
                kb = kb2 + kk
                nc.vector.tensor_copy(xT[kb][:, sl],
                                      ps[:, kk * 128:(kk + 1) * 128])
                nc.vector.tensor_scalar(xTn[kb][:, sl], xT[kb][:, sl],
                                        -1.0, None, ALU.mult)

        # ---- pre-loop: only what region (0,0)'s first groups need ----
        with tc.tile_pool(name="tpsum", bufs=2, space="PSUM") as tpsum:
            def pre_ps():
                return tpsum.tile([128, 1024], DT.float32, tag="tp", name="tp")
            do_t_chunk(0, 0, pre_ps)           # tT[0][:, 0:512]
            do_x_piece(0, 0, pre_ps)           # xT/xTn[:, 0:128] kb 0-3
            do_t_chunk(0, 1, pre_ps)
            do_x_piece(0, 4, pre_ps)           # xT/xTn kb 4-7
            do_t_chunk(0, 2, pre_ps)
            do_t_chunk(0, 3, pre_ps)

        # ---- main pools ----
        spool = ctx.enter_context(tc.tile_pool(name="spsum", bufs=3, space="PSUM"))
        dnpool = ctx.enter_context(tc.tile_pool(name="dnpsum", bufs=1, space="PSUM"))
        e_pool = ctx.enter_context(tc.tile_pool(name="epool", bufs=6))
        th_pool = ctx.enter_context(tc.tile_pool(name="thpool", bufs=6))
        z_pool = ctx.enter_context(tc.tile_pool(name="zpool", bufs=8))
        p_pool = ctx.enter_context(tc.tile_pool(name="ppool", bufs=8))
        tail_pool = ctx.enter_context(tc.tile_pool(name="tail", bufs=2))

        def ring_ps():
            return spool.tile([128, 1024], DT.float32, tag="s", name="s")

        # region-0 just-in-time pieces: tT[g] must be emitted before group
        # g's matmuls (one piece at the top of each early group)
        jit_pieces = [lambda kb=kb: do_t_chunk(0, kb, ring_ps)
                      for kb in range(4, KB)]
        jit_pieces.reverse()

        # late prologue pieces, one per pair of main-loop groups
        late_pieces = []
        for i in range(1, NT):
            for kb2 in (0, 4):
                late_pieces.append(
                    lambda i=i, kb2=kb2: do_x_piece(i, kb2, ring_ps))
        for hq in range(4, HB, 4):
            for kb in range(KB):
                late_pieces.append(
                    lambda hq=hq, kb=kb: do_t_chunk(hq, kb, ring_ps))
        late_pieces.reverse()

        pending = []

        def emit_pending(keep):
            while len(pending) > keep:
                pending.pop(0)()

        for j in range(NJ):
            for i in range(NT):
                r = j * NT + i
                dn = dnpool.tile([128, 1024], DT.float32, tag="dn")
                den = dn[:, 0:512]
                num = dn[:, 512:1024]

                # per-slot form: True -> q-form (mh=s*e DVE, q=th*mh Pool,
                # 3 num mms, no S); False -> z-form (z=e2-th, p=s*z DVE,
                # 2 num mms + S-correction matmul for that k-block)
                forms = [QFORM[(r * 8 + g) % len(QFORM)] for g in range(8)]

                def s_task(j=j, i=i, num=num, forms=forms):
                    # executes after red_task(g=0) (which carries num's
                    # start=True) and before red_task(g=7) (num's stop);
                    # only z-form k-blocks need the -S correction
                    for kb in range(KB):
                        if forms[kb]:
                            continue
                        nc.tensor.matmul(
                            num,
                            xTn[kb][:, i * 128:(i + 1) * 128],
                            tT[kb][:, j * 512:(j + 1) * 512],
                            start=False, stop=False,
                            skip_group_check=True,
                        )

                for g in range(8):
                    if jit_pieces:
                        jit_pieces.pop()()
                    # emit deferred reductions BEFORE this group's matmuls so
                    # they aren't FIFO-blocked behind a slot-starved matmul
                    emit_pending(LAG)
                    st = spool.tile([128, 1024], DT.float32, tag="s")
                    for widx in range(2):
                        w = g * 2 + widx
                        base = (w % 2) * 64
                        lhsT = xT[w // 2][base:base + 64,
                                          i * 128:(i + 1) * 128]
                        rhs = tT[w // 2][base:base + 64,
                                         j * 512:(j + 1) * 512]
                        nc.tensor.matmul(
                            st[:, widx * 512:(widx + 1) * 512],
                            lhsT, rhs,
                            start=True, stop=True, skip_group_check=True,
                        )

                    e_t = e_pool.tile([128, 1024], DT.float16, tag="e")
                    th_t = th_pool.tile([128, 1024], DT.float16, tag="th")
                    nc.scalar.activation(e_t[:], st[:], AF.Exp, bias=ln2_t[:])
                    nc.scalar.activation(th_t[:], st[:], AF.Tanh, scale=0.5)
                    qform = forms[g]
                    if qform:
                        # mh = s*e = (s*0.5)*e2 ; frees slot w/o waiting z
                        p_t = p_pool.tile([128, 1024], DT.float16, tag="p")
                        nc.vector.scalar_tensor_tensor(
                            p_t[:], st[:], 0.5, e_t[:], ALU.mult, ALU.mult,
                        )
                        q_t = z_pool.tile([128, 1024], DT.float16, tag="z")
                        nc.gpsimd.tensor_tensor(q_t[:], th_t[:], p_t[:],
                                                ALU.mult)
                    else:
                        z_t = z_pool.tile([128, 1024], DT.float16, tag="z")
                        nc.vector.tensor_tensor(z_t[:], e_t[:], th_t[:],
                                                ALU.subtract)
                        p_t = p_pool.tile([128, 1024], DT.float16, tag="p")
                        nc.vector.scalar_tensor_tensor(
                            p_t[:], st[:], 1.0, z_t[:], ALU.mult, ALU.mult,
                        )
                        q_t = None

                    def red_task(g=g, den=den, num=num, e_t=e_t, p_t=p_t,
                                 q_t=q_t):
                        # den mms first so den completes (and the tail's
                        # reciprocal can start) before the num stream ends
                        for c in range(2):
                            w = g * 2 + c
                            sl = slice(c * 512, (c + 1) * 512)
                            nc.tensor.matmul(
                                den, eye_h[:], e_t[:, sl],
                                start=(w == 0), stop=(w == NW - 1),
                                skip_group_check=True,
                            )
                        for c in range(2):
                            w = g * 2 + c
                            sl = slice(c * 512, (c + 1) * 512)
                            nc.tensor.matmul(
                                num, eye_h[:], p_t[:, sl],
                                start=(w == 0),
                                stop=(w == NW - 1 and q_t is None),
                                skip_group_check=True,
                            )
                            if q_t is not None:
                                nc.tensor.matmul(
                                    num, eye_h[:], q_t[:, sl],
                                    start=False, stop=(w == NW - 1),
                                    skip_group_check=True,
                                )
                    pending.append(red_task)
                    if g == 1:
                        # lands after red_task(g=0), which carries num's
                        # start=True; before red_task(g=7), which stops it
                        pending.append(s_task)
                    if late_pieces and g % 2 == 1:
                        late_pieces.pop()()

                def tail_task(j=j, i=i, dn=dn, den=den, num=num):
                    r_t = tail_pool.tile([128, 512], DT.float32, tag="r")
                    nc.vector.reciprocal_approx_fast(r_t[:], den)
                    o_t = tail_pool.tile([128, 512], DT.float32, tag="o")
                    nc.vector.tensor_tensor(o_t[:], num, r_t[:], ALU.mult)
                    nc.sync.dma_start(
                        out_d[i * 128:(i + 1) * 128, j * 512:(j + 1) * 512],
                        o_t[:],
                    )
                pending.append(tail_task)
        emit_pending(0)

    nc.compile()
    return nc


_EYE = None
_PROG = None


def _eye_input():
    global _EYE
    if _EYE is None:
        _EYE = np.eye(128, dtype=np.float32)
    return _EYE


def kernel(x: np.ndarray, template_flat: np.ndarray) -> np.ndarray:
    global _PROG
    x = np.ascontiguousarray(x, dtype=np.float32)
    template_flat = np.ascontiguousarray(template_flat, dtype=np.float32)
    assert x.shape == (N_TOTAL, IN_F) and template_flat.shape == (OUT_F, IN_F)
    if _PROG is None:
        _PROG = build_program()
    eye = _eye_input()
    in_maps = [
        {
            "x": x[c * N_SH:(c + 1) * N_SH],
            "template_flat": template_flat,
            "eye": eye,
        }
        for c in range(N_CORES)
    ]
    res = run_bass_kernel_spmd(_PROG, in_maps, core_ids=list(range(N_CORES)))
    return np.concatenate([r["out"] for r in res.results], axis=0)


# revision 5
# speedup vs baseline: 1.0237x; 1.0091x over previous
"""Trainium2 Bass kernel v2 for the dendritic template-gated FFN.

Math (token n, output feature h; W=16 windows of K=64 input features):
    s[n,h,w] = <x[n, w*64:(w+1)*64], template[h, w*64:(w+1)*64]>
    out[n,h] = sum_w softmax_w(s) * silu(s) = [sum_w s*z_w] / [sum_w e_w]
    with z = e^s - sigma(s) (since e^s*silu(s) = s*(e^s - sigma(s))).

Restructuring vs baseline (which was ACT-bound: exp+tanh+2 DVE/Pool products
and 3 PE reduction streams):
    e2  = 2*e^s = Exp(s + ln2)            (ACT pass 1)
    th  = tanh(s/2)                        (ACT pass 2, same table set)
  z-form slots:
    z'' = e2 - th                          (DVE TT fp16 2x)
    p   = s * z''                          (DVE stt, PSUM fp32 x fp16)
    num'' += I@p ; minus S via matmuls of (-x)@t for z-form k-blocks
  q-form slots (Pool gets off-critical-chain work):
    mh  = (s*0.5)*e2 = s*e^s               (DVE stt; frees psum slot early)
    q   = th * mh                          (Pool TT fp16, all SBUF)
    num'' += I@mh + I@q                    (mh(1+th) = e2*silu)
    den'' += I@e2   for all slots;  out = num'' * recip_fast(den'')
since s*z''-s = 2*s*z = e2*silu.  num''=2num, den''=2den -> exact ratio.

Pipeline: psum = 3-slot pool of [128,1024] s tiles + one [128,1024] den|num
tile. Per slot: 2 PE matmuls -> ACT exp -> ACT tanh -> slot-freeing DVE
product -> deferred PE reduction matmuls (LAG slots later, emitted ahead of
the next matmuls so the PE FIFO never head-blocks; den matmuls first so the
tail reciprocal starts early). q-form pattern [T,T,F,F] keeps Pool's slow
TT off the psum-slot-release chain while absorbing ~1/4 of the elementwise
work. Prologue transposes for i>0 / j>0 are spread one piece per group-pair
through the main loop, using the s-slot pool as scratch.

Measured (TimelineSim, per core): 317282 ns vs 327777 ns for the previous
kernel; engine busy ACT ~267us (84%, bottleneck: 2 transcendental passes
over 131072 psum columns are irreducible), DVE ~239us, PE ~210us, Pool
~137us. Max rel err vs fp32 reference: ~6.4e-4.

Sharding: data-parallel over tokens, 512 per NeuronCore x 8 cores.
"""

import numpy as np
from contextlib import ExitStack

import concourse.bass as bass
import concourse.bacc as bacc
import concourse.mybir as mybir
import concourse.tile as tile
from concourse.bass_utils import run_bass_kernel_spmd

AF = mybir.ActivationFunctionType
ALU = mybir.AluOpType
DT = mybir.dt

N_TOTAL = 4096
IN_F = 1024
OUT_F = 2048
WIN = 64
NW = 16
N_CORES = 8
N_SH = N_TOTAL // N_CORES   # 512 tokens per core
QFORM = [True, True, False, False]   # slot form pattern (True -> q-form)
LAG = 4
LN2 = float(np.log(2.0))


def build_program(n_tok=N_SH):
    nc = bacc.Bacc(
        "TRN2",
        target_bir_lowering=False,
        debug=False,
        enable_asserts=False,
        num_devices=N_CORES,
    )
    x_d = nc.dram_tensor("x", [n_tok, IN_F], DT.float32, kind="ExternalInput").ap()
    t_d = nc.dram_tensor(
        "template_flat", [OUT_F, IN_F], DT.float32, kind="ExternalInput"
    ).ap()
    eye_d = nc.dram_tensor("eye", [128, 128], DT.float32, kind="ExternalInput").ap()
    out_d = nc.dram_tensor("out", [n_tok, OUT_F], DT.float32, kind="ExternalOutput").ap()

    NT = n_tok // 128       # 4 token tiles
    NJ = OUT_F // 512       # 4 h chunks
    KB = IN_F // 128        # 8 k-blocks (2 windows each)
    HB = OUT_F // 128       # 16 h blocks of template

    with ExitStack() as ctx:
        tc = ctx.enter_context(tile.TileContext(nc))

        const_pool = ctx.enter_context(tc.tile_pool(name="const", bufs=1))
        eye_t = const_pool.tile([128, 128], DT.float32, tag="eye")
        nc.sync.dma_start(eye_t[:], eye_d[:])
        eye_h = const_pool.tile([128, 128], DT.float16, tag="eyeh")
        nc.vector.tensor_copy(eye_h[:], eye_t[:])
        ln2_t = const_pool.tile([128, 1], DT.float32, tag="ln2")
        nc.gpsimd.memset(ln2_t[:], LN2)

        persist = ctx.enter_context(tc.tile_pool(name="persist", bufs=1))
        xT = [persist.tile([128, n_tok], DT.float16, tag=f"xT{kb}", name=f"xT{kb}")
              for kb in range(KB)]
        xTn = [persist.tile([128, n_tok], DT.float16, tag=f"xTn{kb}", name=f"xTn{kb}")
               for kb in range(KB)]
        tT = [persist.tile([128, OUT_F], DT.float16, tag=f"tT{kb}", name=f"tT{kb}")
              for kb in range(KB)]

        # staging tiles stay open through the main loop (late transposes)
        stage = ctx.enter_context(tc.tile_pool(name="stage", bufs=1))
        t_nm = [stage.tile([128, IN_F], DT.float32, tag=f"tnm{hb}",
                           name=f"tnm{hb}") for hb in range(HB)]
        x_nm = [stage.tile([128, IN_F], DT.float32, tag=f"xnm{i}",
                           name=f"xnm{i}") for i in range(NT)]
        x_re = x_d.rearrange("(i p) k -> p i k", p=128)
        t_re = t_d.rearrange("(h p) k -> p h k", p=128)
        # region (0,0)'s inputs first: t rows 0-511 and x tile 0
        for hb in range(2):
            nc.sync.dma_start(t_nm[hb][:], t_re[:, hb, :])
        nc.sync.dma_start(x_nm[0][:], x_re[:, 0, :])
        for hb in range(2, 4):
            nc.sync.dma_start(t_nm[hb][:], t_re[:, hb, :])
        for i in range(1, NT):
            nc.sync.dma_start(x_nm[i][:], x_re[:, i, :])
        for hb in range(4, HB):
            nc.sync.dma_start(t_nm[hb][:], t_re[:, hb, :])

        # transpose piece emitters; `ps_fn()` yields a [128,1024] psum tile
        def do_t_chunk(hq, kb, ps_fn):
            """template rows hq*128..(hq+4)*128, k-block kb -> tT[kb]."""
            ps = ps_fn()
            for q in range(4):
                nc.tensor.transpose(
                    ps[:, q * 128:(q + 1) * 128],
                    t_nm[hq + q][:, kb * 128:(kb + 1) * 128],
                    eye_t[:],
                )
            nc.vector.tensor_copy(tT[kb][:, hq * 128:(hq + 4) * 128],
                                  ps[:, 0:512])

        def do_x_piece(i, kb2, ps_fn):
            """x token-tile i, k-blocks kb2..kb2+3 -> xT/xTn cols."""
            ps = ps_fn()
            for kk in range(4):
                kb = kb2 + kk
                nc.tensor.transpose(
                    ps[:, kk * 128:(kk + 1) * 128],
                    x_nm[i][:, kb * 128:(kb + 1) * 128],
                    eye_t[:],
                )
            sl = slice(i * 128, (i + 1) * 128)
            for kk in range(4):
                kb = kb2 + kk
                nc.vector.tensor_copy(xT[kb][:, sl],
                                      ps[:, kk * 128:(kk + 1) * 128])
                nc.vector.tensor_scalar(xTn[kb][:, sl], xT[kb][:, sl],
                                        -1.0, None, ALU.mult)

        # ---- pre-loop: only what region (0,0)'s first groups need ----
        with tc.tile_pool(name="tpsum", bufs=2, space="PSUM") as tpsum:
            def pre_ps():
                return tpsum.tile([128, 1024], DT.float32, tag="tp", name="tp")
            do_t_chunk(0, 0, pre_ps)           # tT[0][:, 0:512]
            do_x_piece(0, 0, pre_ps)           # xT/xTn[:, 0:128] kb 0-3
            do_t_chunk(0, 1, pre_ps)
            do_x_piece(0, 4, pre_ps)           # xT/xTn kb 4-7
            do_t_chunk(0, 2, pre_ps)
            do_t_chunk(0, 3, pre_ps)

        # ---- main pools ----
        spool = ctx.enter_context(tc.tile_pool(name="spsum", bufs=3, space="PSUM"))
        dnpool = ctx.enter_context(tc.tile_pool(name="dnpsum", bufs=1, space="PSUM"))
        e_pool = ctx.enter_context(tc.tile_pool(name="epool", bufs=6))
        th_pool = ctx.enter_context(tc.tile_pool(name="thpool", bufs=6))
        z_pool = ctx.enter_context(tc.tile_pool(name="zpool", bufs=8))
        p_pool = ctx.enter_context(tc.tile_pool(name="ppool", bufs=8))
        tail_pool = ctx.enter_context(tc.tile_pool(name="tail", bufs=2))

        def ring_ps():
            return spool.tile([128, 1024], DT.float32, tag="s", name="s")

        # region-0 just-in-time pieces: tT[g] must be emitted before group
        # g's matmuls (one piece at the top of each early group)
        jit_pieces = [lambda kb=kb: do_t_chunk(0, kb, ring_ps)
                      for kb in range(4, KB)]
        jit_pieces.reverse()

        # late prologue pieces, one per pair of main-loop groups
        late_pieces = []
        for i in range(1, NT):
            for kb2 in (0, 4):
                late_pieces.append(
                    lambda i=i, kb2=kb2: do_x_piece(i, kb2, ring_ps))
        for hq in range(4, HB, 4):
            for kb in range(KB):
                late_pieces.append(
                    lambda hq=hq, kb=kb: do_t_chunk(hq, kb, ring_ps))
        late_pieces.reverse()

        pending = []

        def emit_pending(keep):
            while len(pending) > keep:
                pending.pop(0)()

        for j in range(NJ):
            for i in range(NT):
                r = j * NT + i
                dn = dnpool.tile([128, 1024], DT.float32, tag="dn")
                den = dn[:, 0:512]
                num = dn[:, 512:1024]

                # per-slot form: True -> q-form (mh=s*e DVE, q=th*mh Pool,
                # 3 num mms, no S); False -> z-form (z=e2-th, p=s*z DVE,
                # 2 num mms + S-correction matmul for that k-block)
                forms = [QFORM[(r * 8 + g) % len(QFORM)] for g in range(8)]

                def s_task(j=j, i=i, num=num, forms=forms):
                    # executes after red_task(g=0) (which carries num's
                    # start=True) and before red_task(g=7) (num's stop);
                    # only z-form k-blocks need the -S correction
                    for kb in range(KB):
                        if forms[kb]:
                            continue
                        nc.tensor.matmul(
                            num,
                            xTn[kb][:, i * 128:(i + 1) * 128],
                            tT[kb][:, j * 512:(j + 1) * 512],
                            start=False, stop=False,
                            skip_group_check=True,
                        )

                for g in range(8):
                    if jit_pieces:
                        jit_pieces.pop()()
                    # emit deferred reductions BEFORE this group's matmuls so
                    # they aren't FIFO-blocked behind a slot-starved matmul
                    emit_pending(LAG)
                    st = spool.tile([128, 1024], DT.float32, tag="s")
                    for widx in range(2):
                        w = g * 2 + widx
                        base = (w % 2) * 64
                        lhsT = xT[w // 2][base:base + 64,
                                          i * 128:(i + 1) * 128]
                        rhs = tT[w // 2][base:base + 64,
                                         j * 512:(j + 1) * 512]
                        nc.tensor.matmul(
                            st[:, widx * 512:(widx + 1) * 512],
                            lhsT, rhs,
                            start=True, stop=True, skip_group_check=True,
                        )

                    e_t = e_pool.tile([128, 1024], DT.float16, tag="e")
                    th_t = th_pool.tile([128, 1024], DT.float16, tag="th")
                    nc.scalar.activation(e_t[:], st[:], AF.Exp, bias=ln2_t[:])
                    nc.scalar.activation(th_t[:], st[:], AF.Tanh, scale=0.5)
                    qform = forms[g]
                    if qform:
                        # mh = s*e = (s*0.5)*e2 ; frees slot w/o waiting z
                        p_t = p_pool.tile([128, 1024], DT.float16, tag="p")
                        nc.vector.scalar_tensor_tensor(
                            p_t[:], st[:], 0.5, e_t[:], ALU.mult, ALU.mult,
                        )
                        q_t = z_pool.tile([128, 1024], DT.float16, tag="z")
                        nc.gpsimd.tensor_tensor(q_t[:], th_t[:], p_t[:],
                                                ALU.mult)
                    else:
                        z_t = z_pool.tile([128, 1024], DT.float16, tag="z")
                        nc.vector.tensor_tensor(z_t[:], e_t[:], th_t[:],
                                                ALU.subtract)
                        p_t = p_pool.tile([128, 1024], DT.float16, tag="p")
                        nc.vector.scalar_tensor_tensor(
                            p_t[:], st[:], 1.0, z_t[:], ALU.mult, ALU.mult,
                        )
                        q_t = None

                    def red_task(g=g, den=den, num=num, e_t=e_t, p_t=p_t,
                                 q_t=q_t):
                        # den mms first so den completes (and the tail's
                        # reciprocal can start) before the num stream ends
                        for c in range(2):
                            w = g * 2 + c
                            sl = slice(c * 512, (c + 1) * 512)
                            nc.tensor.matmul(
                                den, eye_h[:], e_t[:, sl],
                                start=(w == 0), stop=(w == NW - 1),
                                skip_group_check=True,
                            )
                        for c in range(2):
                            w = g * 2 + c
                            sl = slice(c * 512, (c + 1) * 512)
                            nc.tensor.matmul(
                                num, eye_h[:], p_t[:, sl],
                                start=(w == 0),
                                stop=(w == NW - 1 and q_t is None),
                                skip_group_check=True,
                            )
                            if q_t is not None:
                                nc.tensor.matmul(
                                    num, eye_h[:], q_t[:, sl],
                                    start=False, stop=(w == NW - 1),
                                    skip_group_check=True,
                                )
                    pending.append(red_task)
                    if g == 1:
                        # lands after red_task(g=0), which carries num's
                        # start=True; before red_task(g=7), which stops it
                        pending.append(s_task)
                    if late_pieces and g % 2 == 1:
                        late_pieces.pop()()

                def tail_task(j=j, i=i, dn=dn, den=den, num=num):
                    r_t = tail_pool.tile([128, 512], DT.float32, tag="r")
                    nc.vector.reciprocal_approx_fast(r_t[:], den)
                    o_t = tail_pool.tile([128, 512], DT.float32, tag="o")
                    nc.vector.tensor_tensor(o_t[:], num, r_t[:], ALU.mult)
                    nc.sync.dma_start(
                        out_d[i * 128:(i + 1) * 128, j * 512:(j + 1) * 512],
                        o_t[:],
                    )
                pending.append(tail_task)
        emit_pending(0)

    nc.compile()
    return nc


_EYE = None
_PROG = None


def _eye_input():
    global _EYE
    if _EYE is None:
        _EYE = np.eye(128, dtype=np.float32)
    return _EYE


def kernel(x: np.ndarray, template_flat: np.ndarray) -> np.ndarray:
    global _PROG
    x = np.ascontiguousarray(x, dtype=np.float32)
    template_flat = np.ascontiguousarray(template_flat, dtype=np.float32)
    assert x.shape == (N_TOTAL, IN_F) and template_flat.shape == (OUT_F, IN_F)
    if _PROG is None:
        _PROG = build_program()
    eye = _eye_input()
    in_maps = [
        {
            "x": x[c * N_SH:(c + 1) * N_SH],
            "template_flat": template_flat,
            "eye": eye,
        }
        for c in range(N_CORES)
    ]
    res = run_bass_kernel_spmd(_PROG, in_maps, core_ids=list(range(N_CORES)))
    return np.concatenate([r["out"] for r in res.results], axis=0)
